# revision 15
# baseline (speedup 1.0000x reference)
"""Trainium2 Bass kernel for nn_MultiHeadAttention (B=2, S=4096, D=512, H=8).

Computes: q/k/v = relu(x@W+b) per head, softmax(q k^T / sqrt(64)) v,
out = relu(concat_heads @ Wo + bo).

Sharding: 8 cores = 2 (batch) x 4 (query-slice).  Each core computes full
K/V projections for its batch (redundant across the 4 q-slice cores) and
attention + output projection for its 1024-row query slice.  No collectives;
the host concatenates the 8 output slices.

All-bf16 "flipped-PV" scheme (v3):
  - Projections and QK scores as in v1 (feature-major lhsT, paired heads in
    PE row groups 0/64, bias+relu fused on DVE, exp on ACT with scale 1/8).
  - The PV matmul is FLIPPED: out[128 queries, 65] with lhsT = pT slice
    [128 keys, 128 q] and rhs = V_pad [128 keys, 64+ones].  Matmul cost is
    the out free dim (65 cycles) and the per-matmul LDWEIGHTS streams 2
    rows/cycle, fully hidden - so PV drops from 512 to ~65 cycles per
    (ktile, head, qtile): 262k -> 133k PE cycles total.
  - Four [128, 65] accumulator regions share one PSUM bank per head
    (psU [128, 4, 65]); hardware start=True zeroing is bank-granular, so
    the bank is DVE-memset once per block and all PV matmuls accumulate
    with start=False (skip_group_check).
  - The ones column makes psU[:, qt, 64] the softmax denominator per query
    IN THE QUERY PARTITION: normalize is one tiny DVE reciprocal [128, 4]
    plus per-qtile tensor_scalar multiplies - no [1,512] reciprocal, no
    gpsimd broadcast, no deferred normalize queue.
  - O lands query-major; a PE transpose (identity matmul) per [128, 128]
    block rebuilds feature-major O^T for the output projection.  qc0's
    transposes+outproj run as fillers inside the last attention block.
"""

import os

import numpy as np
import ml_dtypes

import concourse.bass as bass
import concourse.mybir as mybir
import concourse.tile as tile
from concourse import bacc
from concourse import bass_utils

F32 = mybir.dt.float32
BF16 = mybir.dt.bfloat16
AF = mybir.ActivationFunctionType
ALU = mybir.AluOpType

P = 128
D = 512
H = 8
DH = 64
DT = D // P  # 4 (also = number of head pairs)
B = 2
S = 4096
NCORES = 8
QSPLIT = 4
SQ_FULL = S // QSPLIT  # 1024 query rows per core
QC = 512               # q-chunk (matmul free dim / PSUM bank width)
QT_C = QC // P         # 4 qtiles per q-chunk


def build_mha(sk=S, sq=SQ_FULL, skip_vbias=False):
    """Build the SPMD Bass program (identical on all cores)."""
    nc = bacc.Bacc("TRN2", target_bir_lowering=False, debug=False,
                   num_devices=NCORES)

    xT_d = nc.dram_tensor("xT_bf", (P, DT * sk), BF16,
                          kind="ExternalInput").ap()  # chunk-major, see prep
    xqT_d = nc.dram_tensor("xqT_bf", (P, DT * sq), BF16,
                           kind="ExternalInput").ap()
    w_dram = {}
    for n in ("wq", "wk", "wv", "wo"):
        w_dram[n] = nc.dram_tensor(n, (P, DT * D), BF16,
                                   kind="ExternalInput").ap()
    b_dram = {
        "bq": nc.dram_tensor("bq", (P, DT), F32, kind="ExternalInput").ap(),
        "bk": nc.dram_tensor("bk", (P, DT), F32, kind="ExternalInput").ap(),
        "bv": nc.dram_tensor("bv", (1, D), BF16, kind="ExternalInput").ap(),
        "bo": nc.dram_tensor("bo", (1, D), BF16, kind="ExternalInput").ap(),
    }
    ident_d = nc.dram_tensor("ident", (P, P), BF16, kind="ExternalInput").ap()
    out = nc.dram_tensor("out", (sq, D), F32, kind="ExternalOutput").ap()

    with tile.TileContext(nc) as tc:
        _build_tile(tc, xT_d, xqT_d, w_dram, b_dram, ident_d, out, sk, sq,
                    skip_vbias)

    nc.compile()
    return nc


def _build_tile(tc, xT_d, xqT_d, w_dram, b_dram, ident_d, out, sk, sq,
                skip_vbias=False):
    nc = tc.nc
    SK_T = sk // P            # ktiles of the key/value sequence
    SQ_T = sq // P
    NQC = sq // QC            # q chunks per core
    CH = min(4, SK_T)         # stiles per projection chunk
    NCH = SK_T // CH

    with (
        tc.tile_pool(name="singles", bufs=1) as singles,
        tc.tile_pool(name="work", bufs=3) as work,
        tc.tile_pool(name="psum", bufs=2, space="PSUM") as psum,
    ):
        # ---- startup: only what Q-proj pair 0 needs, first ----
        w_bf = {}
        w_bf["wq"] = singles.tile([P, DT, D], BF16, name="wq_bf")
        nc.sync.dma_start(w_bf["wq"], w_dram["wq"].rearrange(
            "p (t n) -> p t n", t=DT))
        b_col = {}
        b_col["bq"] = singles.tile([P, DT], F32, name="bq_col")
        nc.sync.dma_start(b_col["bq"], b_dram["bq"])
        xTq = singles.tile([P, DT, sq], BF16)
        nc.sync.dma_start(xTq, xqT_d.rearrange("p (t s) -> p t s", t=DT))

        QT = singles.tile([P, DT, sq], BF16)

        def qproj(j, nq):
            psQ = psum.tile([P, QC], F32, tag="proj", bufs=1, name="psQ")
            for kt in range(DT):
                nc.tensor.matmul(
                    psQ, w_bf["wq"][:, kt, j * P:(j + 1) * P],
                    xTq[:, kt, nq * QC:(nq + 1) * QC],
                    start=(kt == 0), stop=(kt == DT - 1))
            nc.vector.tensor_scalar(
                QT[:, j, nq * QC:(nq + 1) * QC], psQ,
                b_col["bq"][:, j:j + 1], 0.0, op0=ALU.add, op1=ALU.max)

        qproj(0, 0)
        if NQC > 1:
            qproj(0, 1)

        # ---- K-proj deps next (attention can start before V exists) ----
        b_row = {}
        w_bf["wk"] = singles.tile([P, DT, D], BF16, name="wk_bf")
        nc.sync.dma_start(w_bf["wk"], w_dram["wk"].rearrange(
            "p (t n) -> p t n", t=DT))
        b_col["bk"] = singles.tile([P, DT], F32, name="bk_col")
        nc.sync.dma_start(b_col["bk"], b_dram["bk"])
        CHP = CH * P
        xT = singles.tile([P, NCH, DT, CHP], BF16)
        xT_src = xT_d.rearrange("p (n t s) -> p n t s", n=NCH, t=DT)
        nc.sync.dma_start(xT[:, 0], xT_src[:, 0])
        for n in ("wv", "wo"):
            wb = singles.tile([P, DT, D], BF16, name=f"{n}_bf")
            nc.sync.dma_start(wb, w_dram[n].rearrange(
                "p (t n) -> p t n", t=DT))
            w_bf[n] = wb
            if n == "wv":
                br = singles.tile([1, D], BF16, name="bv_row")
                nc.sync.dma_start(br, b_dram["bv"])
                b_row["bv"] = br
        br = singles.tile([1, D], BF16, name="bo_row")
        nc.sync.dma_start(br, b_dram["bo"])
        b_row["bo"] = br
        ident = singles.tile([P, P], BF16, name="ident")
        nc.sync.dma_start(ident, ident_d)

        # ---- persistent SBUF tensors ----
        xT1 = singles.tile([1, sk], BF16)
        nc.vector.memset(xT1, 1.0)
        KT = singles.tile([P, DT, sk], BF16)
        V_pad = singles.tile([P, SK_T, H, DH + 1], BF16)
        nc.vector.memset(V_pad[:, :, :, DH:DH + 1], 1.0)
        O_nat = singles.tile([P, SQ_T, H, DH], BF16)   # query-major O
        OT = singles.tile([P, DT, sq], BF16)           # feature-major O^T
        OT1 = singles.tile([1, sq], BF16)
        nc.vector.memset(OT1, 1.0)

        # PSUM: proj 2x1 + scores 2x2 + psU 2x(1040B) + tpose 1x(256B)
        def vproj(st):
            n, si = st // CH, st % CH
            psV = psum.tile([P, D], F32, tag="proj", bufs=1, name="psV")
            for kt in range(DT):
                nc.tensor.matmul(
                    psV, xT[:, n, kt, si * P:(si + 1) * P],
                    w_bf["wv"][:, kt, :],
                    start=(kt == 0),
                    stop=(skip_vbias and kt == DT - 1))
            if not skip_vbias:
                nc.tensor.matmul(psV, xT1[:, st * P:(st + 1) * P],
                                 b_row["bv"], start=False, stop=True)
            nc.vector.tensor_scalar_max(
                V_pad[:, st, :, 0:DH],
                psV.rearrange("p (h d) -> p h d", h=H), 0.0)

        def kproj(j, n):
            psK = psum.tile([P, CH * P], F32, tag="proj", bufs=1, name="psK")
            for kt in range(DT):
                nc.tensor.matmul(
                    psK, w_bf["wk"][:, kt, j * P:(j + 1) * P],
                    xT[:, n, kt, :],
                    start=(kt == 0), stop=(kt == DT - 1))
            nc.vector.tensor_scalar(
                KT[:, j, n * CH * P:(n + 1) * CH * P], psK,
                b_col["bk"][:, j:j + 1], 0.0, op0=ALU.add, op1=ALU.max)

        def attn_qk_exp(j, qc, kt, pt_tag="pT", pt_bufs=5):
            """Scores + exp for one ktile x 2 heads -> one ACT op."""
            q0 = qc * QC
            psS = psum.tile([P, 2 * QC], F32, tag="scores", bufs=2,
                            name="psS")
            nc.tensor.matmul(
                psS[:, 0:QC],
                KT[0:DH, j, kt * P:(kt + 1) * P],
                QT[0:DH, j, q0:q0 + QC], start=True, stop=True)
            nc.tensor.matmul(
                psS[:, QC:2 * QC],
                KT[DH:P, j, kt * P:(kt + 1) * P],
                QT[DH:P, j, q0:q0 + QC], start=True, stop=True)
            pT = work.tile([P, 2 * QC], BF16, tag=pt_tag,
                           bufs=pt_bufs, name="pT")
            nc.scalar.activation(pT, psS, AF.Exp, scale=0.125)
            return pT

        def attn_u(j, kt, pT, psU_A, psU_B):
            """Flipped PV: out[128 q, 65] per (head, qtile); start=False
            always (bank pre-zeroed by DVE), 65-cycle matmuls with hidden
            per-matmul LDWEIGHTS."""
            last = kt == SK_T - 1
            for h, psU in ((0, psU_A), (1, psU_B)):
                for qt in range(QT_C):
                    nc.tensor.matmul(
                        psU[:, qt, :],
                        pT[:, h * QC + qt * P:h * QC + (qt + 1) * P],
                        V_pad[:, kt, 2 * j + h, :],
                        start=False, stop=last, skip_group_check=True)

        def new_psU():
            """Two bank-packed accumulators (one per head), DVE-zeroed."""
            a = psum.tile([P, QT_C, DH + 1], F32, tag="psU", name="psU_A")
            b = psum.tile([P, QT_C, DH + 1], F32, tag="psU", name="psU_B")
            nc.vector.memset(a, 0.0)
            nc.vector.memset(b, 0.0)
            return (a, b)

        def finish_block(j, qc, psU):
            """Normalize straight out of PSUM: reciprocal of the 4 denom
            columns (per query partition!), then per-qtile scale into
            query-major O."""
            for h, psUh in enumerate(psU):
                rcp = work.tile([P, QT_C, 1], F32, tag="rcp", bufs=4,
                                name="rcp")
                nc.vector.reciprocal(rcp, psUh[:, :, DH:DH + 1])
                for qt in range(QT_C):
                    nc.vector.tensor_scalar(
                        O_nat[:, qc * QT_C + qt, 2 * j + h, :],
                        psUh[:, qt, 0:DH], rcp[:, qt], None, op0=ALU.mult)

        def attn_group(j, qc, kt, psU_A, psU_B):
            pT = attn_qk_exp(j, qc, kt)
            attn_u(j, kt, pT, psU_A, psU_B)

        def attn_span(j, qc, kts, psU, fillers=(), precomputed=()):
            """Emit the kt groups of one attention block, sprinkling
            `fillers` between groups so the in-order PE/DVE do them inside
            this ACT-bound stretch."""
            fillers = list(fillers)
            for kt, pT in precomputed:
                attn_u(j, kt, pT, psU[0], psU[1])
            ngroups = len(kts)
            spacing = max(1, ngroups // (len(fillers) + 1))
            gi = 0
            for kt in kts:
                attn_group(j, qc, kt, psU[0], psU[1])
                gi += 1
                if fillers and gi % spacing == 0:
                    fillers.pop(0)()
            for f in fillers:
                f()
            if kts[-1] == SK_T - 1:
                finish_block(j, qc, psU)

        def transpose_qt(qt):
            """O_nat[:, qt] (query-major) -> OT columns via PE transpose."""
            for j in range(DT):
                tp = psum.tile([P, P], BF16, tag="tpose", bufs=1, name="tp")
                nc.tensor.transpose(tp, O_nat[:, qt, 2 * j:2 * j + 2, :],
                                    ident)
                nc.vector.tensor_copy(OT[:, j, qt * P:(qt + 1) * P], tp)

        def outproj(qt):
            psO = psum.tile([P, D], F32, tag="proj", bufs=1, name="psO")
            nc.tensor.matmul(psO, OT1[:, qt * P:(qt + 1) * P],
                             b_row["bo"], start=True, stop=False)
            for j in range(DT):
                nc.tensor.matmul(psO, OT[:, j, qt * P:(qt + 1) * P],
                                 w_bf["wo"][:, j, :],
                                 start=False, stop=(j == DT - 1))
            o_sb = work.tile([P, D], F32, tag="osb", bufs=2, name="o_sb")
            nc.scalar.activation(o_sb, psO, AF.Relu)
            nc.sync.dma_start(out[qt * P:(qt + 1) * P, :], o_sb)

        # ---- chunk loop: x load + V proj + K proj(pair 0) + attn(0, 0) ----
        psU0 = new_psU()
        N_STORE = 8
        store01 = []
        for n in range(NCH):
            if n > 0:
                nc.sync.dma_start(xT[:, n], xT_src[:, n])
            kproj(0, n)
            kts = list(range(n * CH, (n + 1) * CH))
            pTs = [(kt, attn_qk_exp(0, 0, kt)) for kt in kts]
            for st in kts:
                vproj(st)
            for kt, pT in pTs:
                attn_u(0, kt, pT, psU0[0], psU0[1])
            if NQC > 1 and n < N_STORE:
                store01.append((n, attn_qk_exp(0, 1, n, pt_tag="pT01",
                                               pt_bufs=N_STORE)))
            if kts[-1] == SK_T - 1:
                finish_block(0, 0, psU0)

        # ---- remaining attention with projection fillers ----
        blocks = [(0, qc) for qc in range(1, NQC)]
        blocks += [(j, qc) for j in range(1, DT) for qc in range(NQC)]
        owed = {blk: [] for blk in blocks}
        for (j, qc) in blocks:
            if (j, qc) != (0, 1):
                owed[(j, qc)].append(lambda j=j, qc=qc: qproj(j, qc))
            if qc == 0 and j >= 1:
                for n in range(NCH):
                    owed[(j, qc)].append(lambda j=j, n=n: kproj(j, n))
        for f in owed[blocks[0]]:
            f()
        for bi, (j, qc) in enumerate(blocks):
            fillers = []
            if bi + 1 < len(blocks):
                fillers += owed[blocks[bi + 1]]
            last = bi == len(blocks) - 1
            if last and NQC > 1:
                # qc0's O is complete after block (DT-1, 0): transpose it
                # and run its output projection inside this last block
                for qt in range(SQ_T // NQC):
                    fillers.append(lambda qt=qt: transpose_qt(qt))
                    fillers.append(lambda qt=qt: outproj(qt))
            psU = new_psU()
            if (j, qc) == (0, 1) and store01:
                attn_span(j, qc, list(range(len(store01), SK_T)), psU,
                          fillers, precomputed=store01)
            else:
                attn_span(j, qc, list(range(SK_T)), psU, fillers)

        # ---- tail: last q-chunk's transposes + output rows ----
        qt_lo = SQ_T // NQC if NQC > 1 else 0
        for qt in range(qt_lo, SQ_T):
            transpose_qt(qt)
            outproj(qt)


_NC_CACHE = {}


def _get_nc(sk=S, sq=SQ_FULL, skip_vbias=False):
    key = (sk, sq, skip_vbias)
    if key not in _NC_CACHE:
        _NC_CACHE[key] = build_mha(sk, sq, skip_vbias)
    return _NC_CACHE[key]


def _tile_rows(a):
    """[D, n] -> SBUF layout [P, DT*n]: partition p gets rows p, 128+p, ..."""
    Dd, n = a.shape
    t = Dd // P
    return np.ascontiguousarray(
        a.reshape(t, P, n).transpose(1, 0, 2).reshape(P, t * n))


def _tile_chunks(a, chp):
    """[D, sk] -> chunk-major SBUF layout [P, NCH*DT*chp]."""
    Dd, sk = a.shape
    t, nch = Dd // P, sk // chp
    return np.ascontiguousarray(
        a.reshape(t, P, nch, chp).transpose(1, 2, 0, 3).reshape(P, -1))


def prep_inputs(x, Wq, bq, Wk, bk, Wv, bv, Wo, bo):
    """Host-side sharding/layout prep: bf16 casts, feature-major transpose,
    SBUF pre-tiling.  Returns the 8 per-core input maps."""
    bf = ml_dtypes.bfloat16
    x = np.asarray(x, dtype=np.float32)
    shared = {
        "wq": _tile_rows(np.asarray(Wq, np.float32).astype(bf)),
        "wk": _tile_rows(np.asarray(Wk, np.float32).astype(bf)),
        "wv": _tile_rows(np.asarray(Wv, np.float32).astype(bf)),
        "wo": _tile_rows(np.asarray(Wo, np.float32).astype(bf)),
        "bq": np.ascontiguousarray(
            np.asarray(bq, np.float32).reshape(DT, P).T),
        "bk": np.ascontiguousarray(
            np.asarray(bk, np.float32).reshape(DT, P).T),
        "bv": np.asarray(bv, np.float32).astype(bf).reshape(1, D),
        "bo": np.asarray(bo, np.float32).astype(bf).reshape(1, D),
        "ident": np.eye(P, dtype=np.float32).astype(bf),
    }
    xT_b = [x[b].T.astype(bf) for b in range(B)]
    xT_tiled = [_tile_chunks(xb, 4 * P) for xb in xT_b]
    in_maps = []
    for c in range(NCORES):
        b, qo = divmod(c, QSPLIT)
        m = dict(shared)
        m["xT_bf"] = xT_tiled[b]
        m["xqT_bf"] = _tile_rows(
            xT_b[b][:, qo * SQ_FULL:(qo + 1) * SQ_FULL])
        in_maps.append(m)
    return in_maps


def kernel(x, Wq, bq, Wk, bk, Wv, bv, Wo, bo, **run_kwargs):
    """Full-input entry point: shards across 8 NeuronCores, returns full out."""
    in_maps = prep_inputs(x, Wq, bq, Wk, bk, Wv, bv, Wo, bo)
    nc = _get_nc(skip_vbias=bool(np.all(np.asarray(bv) == 0)))
    res = bass_utils.run_bass_kernel_spmd(
        nc, in_maps, core_ids=list(range(NCORES)), **run_kwargs)
    full = np.empty((B, S, D), np.float32)
    for c in range(NCORES):
        b, qo = divmod(c, QSPLIT)
        full[b, qo * SQ_FULL:(qo + 1) * SQ_FULL] = res.results[c]["out"]
    if run_kwargs:
        return full, res
    return full


# revision 16
# speedup vs baseline: 1.1638x; 1.1638x over previous
"""Trainium2 Bass kernel for nn_MultiHeadAttention (B=2, S=4096, D=512, H=8).

Computes: q/k/v = relu(x@W+b) per head, softmax(q k^T / sqrt(64)) v,
out = relu(concat_heads @ Wo + bo).

Sharding: 8 cores = 2 (batch) x 4 (query-slice).  Each core computes full
K/V projections for its batch (redundant across the 4 q-slice cores) and
attention + output projection for its 1024-row query slice.  No collectives;
the host concatenates the 8 output slices.

All-bf16 "flipped-PV" scheme (v3):
  - Projections and QK scores as in v1 (feature-major lhsT, paired heads in
    PE row groups 0/64, bias+relu fused on DVE, exp on ACT with scale 1/8).
  - The PV matmul is FLIPPED: out[128 queries, 65] with lhsT = pT slice
    [128 keys, 128 q] and rhs = V_pad [128 keys, 64+ones].  Matmul cost is
    the out free dim (65 cycles) and the per-matmul LDWEIGHTS streams 2
    rows/cycle, fully hidden - so PV drops from 512 to ~65 cycles per
    (ktile, head, qtile): 262k -> 133k PE cycles total.
  - Four [128, 65] accumulator regions share one PSUM bank per head
    (psU [128, 4, 65]); hardware start=True zeroing is bank-granular, so
    the bank is DVE-memset once per block and all PV matmuls accumulate
    with start=False (skip_group_check).
  - The ones column makes psU[:, qt, 64] the softmax denominator per query
    IN THE QUERY PARTITION: normalize is one tiny DVE reciprocal [128, 4]
    plus per-qtile tensor_scalar multiplies - no [1,512] reciprocal, no
    gpsimd broadcast, no deferred normalize queue.
  - O lands query-major; a PE transpose (identity matmul) per [128, 128]
    block rebuilds feature-major O^T for the output projection.  qc0's
    transposes+outproj run as fillers inside the last attention block.
"""

import os

import numpy as np
import ml_dtypes

import concourse.bass as bass
import concourse.mybir as mybir
import concourse.tile as tile
from concourse import bacc
from concourse import bass_utils

F32 = mybir.dt.float32
BF16 = mybir.dt.bfloat16
AF = mybir.ActivationFunctionType
ALU = mybir.AluOpType

P = 128
D = 512
H = 8
DH = 64
DT = D // P  # 4 (also = number of head pairs)
B = 2
S = 4096
NCORES = 8
QSPLIT = 4
SQ_FULL = S // QSPLIT  # 1024 query rows per core
QC = 512               # q-chunk (matmul free dim / PSUM bank width)
QT_C = QC // P         # 4 qtiles per q-chunk


def build_mha(sk=S, sq=SQ_FULL, skip_vbias=False):
    """Build the SPMD Bass program (identical on all cores)."""
    nc = bacc.Bacc("TRN2", target_bir_lowering=False, debug=False,
                   num_devices=NCORES)

    xT_d = nc.dram_tensor("xT_bf", (P, DT * sk), BF16,
                          kind="ExternalInput").ap()  # chunk-major, see prep
    xqT_d = nc.dram_tensor("xqT_bf", (P, DT * sq), BF16,
                           kind="ExternalInput").ap()
    w_dram = {}
    for n in ("wq", "wk", "wv", "wo"):
        w_dram[n] = nc.dram_tensor(n, (P, DT * D), BF16,
                                   kind="ExternalInput").ap()
    b_dram = {
        "bq": nc.dram_tensor("bq", (P, DT), F32, kind="ExternalInput").ap(),
        "bk": nc.dram_tensor("bk", (P, DT), F32, kind="ExternalInput").ap(),
        "bv": nc.dram_tensor("bv", (1, D), BF16, kind="ExternalInput").ap(),
        "bo": nc.dram_tensor("bo", (1, D), BF16, kind="ExternalInput").ap(),
    }
    ident_d = nc.dram_tensor("ident", (P, P), BF16, kind="ExternalInput").ap()
    out = nc.dram_tensor("out", (sq, D), F32, kind="ExternalOutput").ap()

    with tile.TileContext(nc) as tc:
        _build_tile(tc, xT_d, xqT_d, w_dram, b_dram, ident_d, out, sk, sq,
                    skip_vbias)

    nc.compile()
    return nc


def _build_tile(tc, xT_d, xqT_d, w_dram, b_dram, ident_d, out, sk, sq,
                skip_vbias=False):
    nc = tc.nc
    SK_T = sk // P            # ktiles of the key/value sequence
    SQ_T = sq // P
    NQC = sq // QC            # q chunks per core
    CH = min(4, SK_T)         # stiles per projection chunk
    NCH = SK_T // CH

    with (
        tc.tile_pool(name="singles", bufs=1) as singles,
        tc.tile_pool(name="work", bufs=3) as work,
        tc.tile_pool(name="psum", bufs=2, space="PSUM") as psum,
    ):
        # ---- startup: only what Q-proj pair 0 needs, first ----
        w_bf = {}
        w_bf["wq"] = singles.tile([P, DT, D], BF16, name="wq_bf")
        nc.sync.dma_start(w_bf["wq"], w_dram["wq"].rearrange(
            "p (t n) -> p t n", t=DT))
        b_col = {}
        b_col["bq"] = singles.tile([P, DT], F32, name="bq_col")
        nc.sync.dma_start(b_col["bq"], b_dram["bq"])
        w_bf["wk"] = singles.tile([P, DT, D], BF16, name="wk_bf")
        nc.sync.dma_start(w_bf["wk"], w_dram["wk"].rearrange(
            "p (t n) -> p t n", t=DT))
        CHP = CH * P
        NCH_ = (sk // P) // CH
        xT = singles.tile([P, NCH_, DT, CHP], BF16)
        xT_src = xT_d.rearrange("p (n t s) -> p n t s", n=NCH_, t=DT)
        nc.sync.dma_start(xT[:, 0], xT_src[:, 0])
        xTq = singles.tile([P, DT, sq], BF16)
        nc.sync.dma_start(xTq, xqT_d.rearrange("p (t s) -> p t s", t=DT))

        QT = singles.tile([P, DT, sq], BF16)

        def qproj(j, nq):
            psQ = psum.tile([P, QC], F32, tag="proj", bufs=1, name="psQ")
            for kt in range(DT):
                nc.tensor.matmul(
                    psQ, w_bf["wq"][:, kt, j * P:(j + 1) * P],
                    xTq[:, kt, nq * QC:(nq + 1) * QC],
                    start=(kt == 0), stop=(kt == DT - 1))
            nc.vector.tensor_scalar(
                QT[:, j, nq * QC:(nq + 1) * QC], psQ,
                b_col["bq"][:, j:j + 1], 0.0, op0=ALU.add, op1=ALU.max)

        qproj(0, 0)
        if NQC > 1:
            qproj(0, 1)

        # ---- K-proj deps next (attention can start before V exists) ----
        b_row = {}
        b_col["bk"] = singles.tile([P, DT], F32, name="bk_col")
        nc.sync.dma_start(b_col["bk"], b_dram["bk"])
        for n in ("wv", "wo"):
            wb = singles.tile([P, DT, D], BF16, name=f"{n}_bf")
            nc.sync.dma_start(wb, w_dram[n].rearrange(
                "p (t n) -> p t n", t=DT))
            w_bf[n] = wb
            if n == "wv":
                br = singles.tile([1, D], BF16, name="bv_row")
                nc.sync.dma_start(br, b_dram["bv"])
                b_row["bv"] = br
        br = singles.tile([1, D], BF16, name="bo_row")
        nc.sync.dma_start(br, b_dram["bo"])
        b_row["bo"] = br
        ident = singles.tile([P, P], BF16, name="ident")
        nc.sync.dma_start(ident, ident_d)

        # ---- persistent SBUF tensors ----
        xT1 = singles.tile([1, sk], BF16)
        nc.vector.memset(xT1, 1.0)
        KT = singles.tile([P, DT, sk], BF16)
        V_pad = singles.tile([P, SK_T, H, DH + 1], BF16)
        nc.vector.memset(V_pad[:, :, :, DH:DH + 1], 1.0)
        O_nat = singles.tile([P, SQ_T, H, DH], BF16)   # query-major O
        OT = singles.tile([P, DT, sq], BF16)           # feature-major O^T
        OT1 = singles.tile([1, sq], BF16)
        nc.vector.memset(OT1, 1.0)

        # PSUM: proj 2x1 + scores 2x2 + psU 2x(1040B) + tpose 1x(256B)
        def vproj(st):
            n, si = st // CH, st % CH
            psV = psum.tile([P, D], F32, tag="proj", bufs=1, name="psV")
            for kt in range(DT):
                nc.tensor.matmul(
                    psV, xT[:, n, kt, si * P:(si + 1) * P],
                    w_bf["wv"][:, kt, :],
                    start=(kt == 0),
                    stop=(skip_vbias and kt == DT - 1))
            if not skip_vbias:
                nc.tensor.matmul(psV, xT1[:, st * P:(st + 1) * P],
                                 b_row["bv"], start=False, stop=True)
            nc.vector.tensor_scalar_max(
                V_pad[:, st, :, 0:DH],
                psV.rearrange("p (h d) -> p h d", h=H), 0.0)

        def kproj(j, n):
            psK = psum.tile([P, CH * P], F32, tag="proj", bufs=1, name="psK")
            for kt in range(DT):
                nc.tensor.matmul(
                    psK, w_bf["wk"][:, kt, j * P:(j + 1) * P],
                    xT[:, n, kt, :],
                    start=(kt == 0), stop=(kt == DT - 1))
            nc.vector.tensor_scalar(
                KT[:, j, n * CH * P:(n + 1) * CH * P], psK,
                b_col["bk"][:, j:j + 1], 0.0, op0=ALU.add, op1=ALU.max)

        def attn_qk_exp(j, qc, kt, pt_tag="pT", pt_bufs=5):
            """Scores + exp for one ktile x 2 heads -> one ACT op."""
            q0 = qc * QC
            psS = psum.tile([P, 2 * QC], F32, tag="scores", bufs=2,
                            name="psS")
            nc.tensor.matmul(
                psS[:, 0:QC],
                KT[0:DH, j, kt * P:(kt + 1) * P],
                QT[0:DH, j, q0:q0 + QC], start=True, stop=True)
            nc.tensor.matmul(
                psS[:, QC:2 * QC],
                KT[DH:P, j, kt * P:(kt + 1) * P],
                QT[DH:P, j, q0:q0 + QC], start=True, stop=True)
            pT = work.tile([P, 2 * QC], BF16, tag=pt_tag,
                           bufs=pt_bufs, name="pT")
            nc.scalar.activation(pT, psS, AF.Exp, scale=0.125)
            return pT

        def attn_u(j, kt, pT, psU_A, psU_B):
            """Flipped PV: out[128 q, 65] per (head, qtile); start=False
            always (bank pre-zeroed by DVE), 65-cycle matmuls with hidden
            per-matmul LDWEIGHTS."""
            last = kt == SK_T - 1
            for h, psU in ((0, psU_A), (1, psU_B)):
                for qt in range(QT_C):
                    nc.tensor.matmul(
                        psU[:, qt, :],
                        pT[:, h * QC + qt * P:h * QC + (qt + 1) * P],
                        V_pad[:, kt, 2 * j + h, :],
                        start=False, stop=last, skip_group_check=True)

        def new_psU():
            """Two bank-packed accumulators (one per head), DVE-zeroed."""
            a = psum.tile([P, QT_C, DH + 1], F32, tag="psU", name="psU_A")
            b = psum.tile([P, QT_C, DH + 1], F32, tag="psU", name="psU_B")
            nc.vector.memset(a, 0.0)
            nc.vector.memset(b, 0.0)
            return (a, b)

        def finish_block(j, qc, psU):
            """Normalize straight out of PSUM: reciprocal of the 4 denom
            columns (per query partition!), then per-qtile scale into
            query-major O."""
            for h, psUh in enumerate(psU):
                rcp = work.tile([P, QT_C, 1], F32, tag="rcp", bufs=4,
                                name="rcp")
                nc.vector.reciprocal(rcp, psUh[:, :, DH:DH + 1])
                for qt in range(QT_C):
                    nc.vector.tensor_scalar(
                        O_nat[:, qc * QT_C + qt, 2 * j + h, :],
                        psUh[:, qt, 0:DH], rcp[:, qt], None, op0=ALU.mult)

        def attn_group(j, qc, kt, psU_A, psU_B):
            pT = attn_qk_exp(j, qc, kt)
            attn_u(j, kt, pT, psU_A, psU_B)

        def attn_span(j, qc, kts, psU, fillers=(), precomputed=()):
            """Emit the kt groups of one attention block, sprinkling
            `fillers` between groups so the in-order PE/DVE do them inside
            this ACT-bound stretch."""
            fillers = list(fillers)
            for kt, pT in precomputed:
                attn_u(j, kt, pT, psU[0], psU[1])
            ngroups = len(kts)
            spacing = max(1, ngroups // (len(fillers) + 1))
            gi = 0
            for kt in kts:
                attn_group(j, qc, kt, psU[0], psU[1])
                gi += 1
                if fillers and gi % spacing == 0:
                    fillers.pop(0)()
            for f in fillers:
                f()
            if kts[-1] == SK_T - 1:
                finish_block(j, qc, psU)

        def transpose_qt(qt):
            """O_nat[:, qt] (query-major) -> OT columns via PE transpose."""
            for j in range(DT):
                tp = psum.tile([P, P], BF16, tag="tpose", bufs=1, name="tp")
                nc.tensor.transpose(tp, O_nat[:, qt, 2 * j:2 * j + 2, :],
                                    ident)
                nc.vector.tensor_copy(OT[:, j, qt * P:(qt + 1) * P], tp)

        def outproj(qt):
            psO = psum.tile([P, D], F32, tag="proj", bufs=1, name="psO")
            nc.tensor.matmul(psO, OT1[:, qt * P:(qt + 1) * P],
                             b_row["bo"], start=True, stop=False)
            for j in range(DT):
                nc.tensor.matmul(psO, OT[:, j, qt * P:(qt + 1) * P],
                                 w_bf["wo"][:, j, :],
                                 start=False, stop=(j == DT - 1))
            o_sb = work.tile([P, D], F32, tag="osb", bufs=2, name="o_sb")
            nc.scalar.activation(o_sb, psO, AF.Relu)
            nc.sync.dma_start(out[qt * P:(qt + 1) * P, :], o_sb)

        # ---- chunk loop: x load + V proj + K proj(pair 0) + attn(0, 0) ----
        psU0 = new_psU()
        N_STORE = 8
        store01 = []
        for n in range(NCH):
            if n > 0:
                nc.sync.dma_start(xT[:, n], xT_src[:, n])
            kproj(0, n)
            kts = list(range(n * CH, (n + 1) * CH))
            pTs = [(kt, attn_qk_exp(0, 0, kt)) for kt in kts]
            for (kt, pT) in pTs:
                vproj(kt)
                attn_u(0, kt, pT, psU0[0], psU0[1])
            if NQC > 1 and n < N_STORE:
                store01.append((n, attn_qk_exp(0, 1, n, pt_tag="pT01",
                                               pt_bufs=N_STORE)))
            if kts[-1] == SK_T - 1:
                finish_block(0, 0, psU0)

        # ---- remaining attention with projection fillers ----
        blocks = [(0, qc) for qc in range(1, NQC)]
        blocks += [(j, qc) for j in range(1, DT) for qc in range(NQC)]
        owed = {blk: [] for blk in blocks}
        for (j, qc) in blocks:
            if (j, qc) != (0, 1):
                owed[(j, qc)].append(lambda j=j, qc=qc: qproj(j, qc))
            if qc == 0 and j >= 1:
                for n in range(NCH):
                    owed[(j, qc)].append(lambda j=j, n=n: kproj(j, n))
        for f in owed[blocks[0]]:
            f()
        for bi, (j, qc) in enumerate(blocks):
            fillers = []
            if bi + 1 < len(blocks):
                fillers += owed[blocks[bi + 1]]
            last = bi == len(blocks) - 1
            if last and NQC > 1:
                # qc0's O is complete after block (DT-1, 0): transpose it
                # and run its output projection inside this last block
                for qt in range(SQ_T // NQC):
                    fillers.append(lambda qt=qt: transpose_qt(qt))
                    fillers.append(lambda qt=qt: outproj(qt))
            psU = new_psU()
            if (j, qc) == (0, 1) and store01:
                attn_span(j, qc, list(range(len(store01), SK_T)), psU,
                          fillers, precomputed=store01)
            else:
                attn_span(j, qc, list(range(SK_T)), psU, fillers)

        # ---- tail: last q-chunk's transposes + output rows ----
        qt_lo = SQ_T // NQC if NQC > 1 else 0
        for qt in range(qt_lo, SQ_T):
            transpose_qt(qt)
            outproj(qt)


_NC_CACHE = {}


def _get_nc(sk=S, sq=SQ_FULL, skip_vbias=False):
    key = (sk, sq, skip_vbias)
    if key not in _NC_CACHE:
        _NC_CACHE[key] = build_mha(sk, sq, skip_vbias)
    return _NC_CACHE[key]


def _tile_rows(a):
    """[D, n] -> SBUF layout [P, DT*n]: partition p gets rows p, 128+p, ..."""
    Dd, n = a.shape
    t = Dd // P
    return np.ascontiguousarray(
        a.reshape(t, P, n).transpose(1, 0, 2).reshape(P, t * n))


def _tile_chunks(a, chp):
    """[D, sk] -> chunk-major SBUF layout [P, NCH*DT*chp]."""
    Dd, sk = a.shape
    t, nch = Dd // P, sk // chp
    return np.ascontiguousarray(
        a.reshape(t, P, nch, chp).transpose(1, 2, 0, 3).reshape(P, -1))


def prep_inputs(x, Wq, bq, Wk, bk, Wv, bv, Wo, bo):
    """Host-side sharding/layout prep: bf16 casts, feature-major transpose,
    SBUF pre-tiling.  Returns the 8 per-core input maps."""
    bf = ml_dtypes.bfloat16
    x = np.asarray(x, dtype=np.float32)
    shared = {
        "wq": _tile_rows(np.asarray(Wq, np.float32).astype(bf)),
        "wk": _tile_rows(np.asarray(Wk, np.float32).astype(bf)),
        "wv": _tile_rows(np.asarray(Wv, np.float32).astype(bf)),
        "wo": _tile_rows(np.asarray(Wo, np.float32).astype(bf)),
        "bq": np.ascontiguousarray(
            np.asarray(bq, np.float32).reshape(DT, P).T),
        "bk": np.ascontiguousarray(
            np.asarray(bk, np.float32).reshape(DT, P).T),
        "bv": np.asarray(bv, np.float32).astype(bf).reshape(1, D),
        "bo": np.asarray(bo, np.float32).astype(bf).reshape(1, D),
        "ident": np.eye(P, dtype=np.float32).astype(bf),
    }
    xT_b = [x[b].T.astype(bf) for b in range(B)]
    xT_tiled = [_tile_chunks(xb, 4 * P) for xb in xT_b]
    in_maps = []
    for c in range(NCORES):
        b, qo = divmod(c, QSPLIT)
        m = dict(shared)
        m["xT_bf"] = xT_tiled[b]
        m["xqT_bf"] = _tile_rows(
            xT_b[b][:, qo * SQ_FULL:(qo + 1) * SQ_FULL])
        in_maps.append(m)
    return in_maps


def kernel(x, Wq, bq, Wk, bk, Wv, bv, Wo, bo, **run_kwargs):
    """Full-input entry point: shards across 8 NeuronCores, returns full out."""
    in_maps = prep_inputs(x, Wq, bq, Wk, bk, Wv, bv, Wo, bo)
    nc = _get_nc(skip_vbias=bool(np.all(np.asarray(bv) == 0)))
    res = bass_utils.run_bass_kernel_spmd(
        nc, in_maps, core_ids=list(range(NCORES)), **run_kwargs)
    full = np.empty((B, S, D), np.float32)
    for c in range(NCORES):
        b, qo = divmod(c, QSPLIT)
        full[b, qo * SQ_FULL:(qo + 1) * SQ_FULL] = res.results[c]["out"]
    if run_kwargs:
        return full, res
    return full


# revision 17
# speedup vs baseline: 1.1737x; 1.0086x over previous
"""Trainium2 Bass kernel for nn_MultiHeadAttention (B=2, S=4096, D=512, H=8).

Computes: q/k/v = relu(x@W+b) per head, softmax(q k^T / sqrt(64)) v,
out = relu(concat_heads @ Wo + bo).

Sharding: 8 cores = 2 (batch) x 4 (query-slice).  Each core computes full
K/V projections for its batch (redundant across the 4 q-slice cores) and
attention + output projection for its 1024-row query slice.  No collectives;
the host concatenates the 8 output slices.

All-bf16 "flipped-PV" scheme (v3):
  - Projections and QK scores as in v1 (feature-major lhsT, paired heads in
    PE row groups 0/64, bias+relu fused on DVE, exp on ACT with scale 1/8).
  - The PV matmul is FLIPPED: out[128 queries, 65] with lhsT = pT slice
    [128 keys, 128 q] and rhs = V_pad [128 keys, 64+ones].  Matmul cost is
    the out free dim (65 cycles) and the per-matmul LDWEIGHTS streams 2
    rows/cycle, fully hidden - so PV drops from 512 to ~65 cycles per
    (ktile, head, qtile): 262k -> 133k PE cycles total.
  - Four [128, 65] accumulator regions share one PSUM bank per head
    (psU [128, 4, 65]); hardware start=True zeroing is bank-granular, so
    the bank is DVE-memset once per block and all PV matmuls accumulate
    with start=False (skip_group_check).
  - The ones column makes psU[:, qt, 64] the softmax denominator per query
    IN THE QUERY PARTITION: normalize is one tiny DVE reciprocal [128, 4]
    plus per-qtile tensor_scalar multiplies - no [1,512] reciprocal, no
    gpsimd broadcast, no deferred normalize queue.
  - O lands query-major; a PE transpose (identity matmul) per [128, 128]
    block rebuilds feature-major O^T for the output projection.  qc0's
    transposes+outproj run as fillers inside the last attention block.
"""

import os

import numpy as np
import ml_dtypes

import concourse.bass as bass
import concourse.mybir as mybir
import concourse.tile as tile
from concourse import bacc
from concourse import bass_utils

F32 = mybir.dt.float32
BF16 = mybir.dt.bfloat16
AF = mybir.ActivationFunctionType
ALU = mybir.AluOpType

P = 128
D = 512
H = 8
DH = 64
DT = D // P  # 4 (also = number of head pairs)
B = 2
S = 4096
NCORES = 8
QSPLIT = 4
SQ_FULL = S // QSPLIT  # 1024 query rows per core
QC = 512               # q-chunk (matmul free dim / PSUM bank width)
QT_C = QC // P         # 4 qtiles per q-chunk


def build_mha(sk=S, sq=SQ_FULL, skip_vbias=False):
    """Build the SPMD Bass program (identical on all cores)."""
    nc = bacc.Bacc("TRN2", target_bir_lowering=False, debug=False,
                   num_devices=NCORES)

    xT_d = nc.dram_tensor("xT_bf", (P, DT * sk), BF16,
                          kind="ExternalInput").ap()  # chunk-major, see prep
    xqT_d = nc.dram_tensor("xqT_bf", (P, DT * sq), BF16,
                           kind="ExternalInput").ap()
    w_dram = {}
    for n in ("wq", "wk", "wv", "wo"):
        w_dram[n] = nc.dram_tensor(n, (P, DT * D), BF16,
                                   kind="ExternalInput").ap()
    b_dram = {
        "bq": nc.dram_tensor("bq", (P, DT), F32, kind="ExternalInput").ap(),
        "bk": nc.dram_tensor("bk", (P, DT), F32, kind="ExternalInput").ap(),
        "bv": nc.dram_tensor("bv", (1, D), BF16, kind="ExternalInput").ap(),
        "bo": nc.dram_tensor("bo", (1, D), BF16, kind="ExternalInput").ap(),
    }
    ident_d = nc.dram_tensor("ident", (P, P), BF16, kind="ExternalInput").ap()
    out = nc.dram_tensor("out", (sq, D), F32, kind="ExternalOutput").ap()

    with tile.TileContext(nc) as tc:
        _build_tile(tc, xT_d, xqT_d, w_dram, b_dram, ident_d, out, sk, sq,
                    skip_vbias)

    nc.compile()
    return nc


def _build_tile(tc, xT_d, xqT_d, w_dram, b_dram, ident_d, out, sk, sq,
                skip_vbias=False):
    nc = tc.nc
    SK_T = sk // P            # ktiles of the key/value sequence
    SQ_T = sq // P
    NQC = sq // QC            # q chunks per core
    CH = min(4, SK_T)         # stiles per projection chunk
    NCH = SK_T // CH

    with (
        tc.tile_pool(name="singles", bufs=1) as singles,
        tc.tile_pool(name="work", bufs=3) as work,
        tc.tile_pool(name="psum", bufs=2, space="PSUM") as psum,
    ):
        # ---- startup: only what Q-proj pair 0 needs, first ----
        w_bf = {}
        w_bf["wq"] = singles.tile([P, DT, D], BF16, name="wq_bf")
        nc.sync.dma_start(w_bf["wq"], w_dram["wq"].rearrange(
            "p (t n) -> p t n", t=DT))
        b_col = {}
        b_col["bq"] = singles.tile([P, DT], F32, name="bq_col")
        nc.sync.dma_start(b_col["bq"], b_dram["bq"])
        xTq = singles.tile([P, DT, sq], BF16)
        xqT_src = xqT_d.rearrange("p (t s) -> p t s", t=DT)
        nc.sync.dma_start(xTq[:, :, 0:QC], xqT_src[:, :, 0:QC])
        w_bf["wk"] = singles.tile([P, DT, D], BF16, name="wk_bf")
        nc.sync.dma_start(w_bf["wk"], w_dram["wk"].rearrange(
            "p (t n) -> p t n", t=DT))
        b_col["bk"] = singles.tile([P, DT], F32, name="bk_col")
        nc.sync.dma_start(b_col["bk"], b_dram["bk"])
        CHP = CH * P
        NCH_ = (sk // P) // CH
        xT = singles.tile([P, NCH_, DT, CHP], BF16)
        xT_src = xT_d.rearrange("p (n t s) -> p n t s", n=NCH_, t=DT)
        nc.sync.dma_start(xT[:, 0], xT_src[:, 0])
        if sq > QC:
            nc.sync.dma_start(xTq[:, :, QC:], xqT_src[:, :, QC:])

        QT = singles.tile([P, DT, sq], BF16)

        def qproj(j, nq):
            psQ = psum.tile([P, QC], F32, tag="proj", bufs=1, name="psQ")
            for kt in range(DT):
                nc.tensor.matmul(
                    psQ, w_bf["wq"][:, kt, j * P:(j + 1) * P],
                    xTq[:, kt, nq * QC:(nq + 1) * QC],
                    start=(kt == 0), stop=(kt == DT - 1))
            nc.vector.tensor_scalar(
                QT[:, j, nq * QC:(nq + 1) * QC], psQ,
                b_col["bq"][:, j:j + 1], 0.0, op0=ALU.add, op1=ALU.max)

        qproj(0, 0)
        if NQC > 1:
            qproj(0, 1)

        # ---- K-proj deps next (attention can start before V exists) ----
        b_row = {}
        for n in ("wv", "wo"):
            wb = singles.tile([P, DT, D], BF16, name=f"{n}_bf")
            nc.sync.dma_start(wb, w_dram[n].rearrange(
                "p (t n) -> p t n", t=DT))
            w_bf[n] = wb
            if n == "wv":
                br = singles.tile([1, D], BF16, name="bv_row")
                nc.sync.dma_start(br, b_dram["bv"])
                b_row["bv"] = br
        br = singles.tile([1, D], BF16, name="bo_row")
        nc.sync.dma_start(br, b_dram["bo"])
        b_row["bo"] = br
        ident = singles.tile([P, P], BF16, name="ident")
        nc.sync.dma_start(ident, ident_d)

        # ---- persistent SBUF tensors ----
        xT1 = singles.tile([1, sk], BF16)
        nc.vector.memset(xT1, 1.0)
        KT = singles.tile([P, DT, sk], BF16)
        V_pad = singles.tile([P, SK_T, H, DH + 1], BF16)
        nc.vector.memset(V_pad[:, :, :, DH:DH + 1], 1.0)
        O_nat = singles.tile([P, SQ_T, H, DH], BF16)   # query-major O
        OT = singles.tile([P, DT, sq], BF16)           # feature-major O^T
        OT1 = singles.tile([1, sq], BF16)
        nc.vector.memset(OT1, 1.0)

        # PSUM: proj 2x1 + scores 2x2 + psU 2x(1040B) + tpose 1x(256B)
        def vproj(st):
            n, si = st // CH, st % CH
            psV = psum.tile([P, D], F32, tag="proj", bufs=1, name="psV")
            for kt in range(DT):
                nc.tensor.matmul(
                    psV, xT[:, n, kt, si * P:(si + 1) * P],
                    w_bf["wv"][:, kt, :],
                    start=(kt == 0),
                    stop=(skip_vbias and kt == DT - 1))
            if not skip_vbias:
                nc.tensor.matmul(psV, xT1[:, st * P:(st + 1) * P],
                                 b_row["bv"], start=False, stop=True)
            nc.vector.tensor_scalar_max(
                V_pad[:, st, :, 0:DH],
                psV.rearrange("p (h d) -> p h d", h=H), 0.0)

        def kproj(j, n):
            psK = psum.tile([P, CH * P], F32, tag="proj", bufs=1, name="psK")
            for kt in range(DT):
                nc.tensor.matmul(
                    psK, w_bf["wk"][:, kt, j * P:(j + 1) * P],
                    xT[:, n, kt, :],
                    start=(kt == 0), stop=(kt == DT - 1))
            nc.vector.tensor_scalar(
                KT[:, j, n * CH * P:(n + 1) * CH * P], psK,
                b_col["bk"][:, j:j + 1], 0.0, op0=ALU.add, op1=ALU.max)

        def attn_qk_exp(j, qc, kt, pt_tag="pT", pt_bufs=5):
            """Scores + exp for one ktile x 2 heads -> one ACT op."""
            q0 = qc * QC
            psS = psum.tile([P, 2 * QC], F32, tag="scores", bufs=2,
                            name="psS")
            nc.tensor.matmul(
                psS[:, 0:QC],
                KT[0:DH, j, kt * P:(kt + 1) * P],
                QT[0:DH, j, q0:q0 + QC], start=True, stop=True)
            nc.tensor.matmul(
                psS[:, QC:2 * QC],
                KT[DH:P, j, kt * P:(kt + 1) * P],
                QT[DH:P, j, q0:q0 + QC], start=True, stop=True)
            pT = work.tile([P, 2 * QC], BF16, tag=pt_tag,
                           bufs=pt_bufs, name="pT")
            nc.scalar.activation(pT, psS, AF.Exp, scale=0.125)
            return pT

        def attn_u(j, kt, pT, psU_A, psU_B):
            """Flipped PV: out[128 q, 65] per (head, qtile); start=False
            always (bank pre-zeroed by DVE), 65-cycle matmuls with hidden
            per-matmul LDWEIGHTS."""
            last = kt == SK_T - 1
            for h, psU in ((0, psU_A), (1, psU_B)):
                for qt in range(QT_C):
                    nc.tensor.matmul(
                        psU[:, qt, :],
                        pT[:, h * QC + qt * P:h * QC + (qt + 1) * P],
                        V_pad[:, kt, 2 * j + h, :],
                        start=False, stop=last, skip_group_check=True)

        def new_psU():
            """Two bank-packed accumulators (one per head), DVE-zeroed."""
            a = psum.tile([P, QT_C, DH + 1], F32, tag="psU", name="psU_A")
            b = psum.tile([P, QT_C, DH + 1], F32, tag="psU", name="psU_B")
            nc.vector.memset(a, 0.0)
            nc.vector.memset(b, 0.0)
            return (a, b)

        def finish_block(j, qc, psU):
            """Normalize straight out of PSUM: reciprocal of the 4 denom
            columns (per query partition!), then per-qtile scale into
            query-major O."""
            for h, psUh in enumerate(psU):
                rcp = work.tile([P, QT_C, 1], F32, tag="rcp", bufs=4,
                                name="rcp")
                nc.vector.reciprocal(rcp, psUh[:, :, DH:DH + 1])
                for qt in range(QT_C):
                    nc.vector.tensor_scalar(
                        O_nat[:, qc * QT_C + qt, 2 * j + h, :],
                        psUh[:, qt, 0:DH], rcp[:, qt], None, op0=ALU.mult)

        def attn_group(j, qc, kt, psU_A, psU_B):
            pT = attn_qk_exp(j, qc, kt)
            attn_u(j, kt, pT, psU_A, psU_B)

        def attn_span(j, qc, kts, psU, fillers=(), precomputed=()):
            """Emit the kt groups of one attention block, sprinkling
            `fillers` between groups so the in-order PE/DVE do them inside
            this ACT-bound stretch."""
            fillers = list(fillers)
            for kt, pT in precomputed:
                attn_u(j, kt, pT, psU[0], psU[1])
            ngroups = len(kts)
            spacing = max(1, ngroups // (len(fillers) + 1))
            gi = 0
            for kt in kts:
                attn_group(j, qc, kt, psU[0], psU[1])
                gi += 1
                if fillers and gi % spacing == 0:
                    fillers.pop(0)()
            for f in fillers:
                f()
            if kts[-1] == SK_T - 1:
                finish_block(j, qc, psU)

        def transpose_qt(qt):
            """O_nat[:, qt] (query-major) -> OT columns via PE transpose."""
            for j in range(DT):
                tp = psum.tile([P, P], BF16, tag="tpose", bufs=1, name="tp")
                nc.tensor.transpose(tp, O_nat[:, qt, 2 * j:2 * j + 2, :],
                                    ident)
                nc.vector.tensor_copy(OT[:, j, qt * P:(qt + 1) * P], tp)

        def outproj(qt, tail=False):
            if tail:
                # scores banks are dead after the last exp: borrow a slot so
                # tail chains double-buffer instead of serializing on the
                # single proj bank
                psO = psum.tile([P, 2 * QC], F32, tag="scores", bufs=2,
                                name="psO_t")[:, 0:D]
            else:
                psO = psum.tile([P, D], F32, tag="proj", bufs=1, name="psO")
            nc.tensor.matmul(psO, OT1[:, qt * P:(qt + 1) * P],
                             b_row["bo"], start=True, stop=False)
            for j in range(DT):
                nc.tensor.matmul(psO, OT[:, j, qt * P:(qt + 1) * P],
                                 w_bf["wo"][:, j, :],
                                 start=False, stop=(j == DT - 1))
            o_sb = work.tile([P, D], F32, tag="osb", bufs=2, name="o_sb")
            nc.scalar.activation(o_sb, psO, AF.Relu)
            nc.sync.dma_start(out[qt * P:(qt + 1) * P, :], o_sb)

        # ---- chunk loop: x load + V proj + K proj(pair 0) + attn(0, 0) ----
        psU0 = new_psU()
        N_STORE = 8
        store01 = []
        for n in range(NCH):
            if n > 0:
                nc.sync.dma_start(xT[:, n], xT_src[:, n])
            kproj(0, n)
            kts = list(range(n * CH, (n + 1) * CH))
            pTs = [(kt, attn_qk_exp(0, 0, kt)) for kt in kts]
            for (kt, pT) in pTs:
                vproj(kt)
                attn_u(0, kt, pT, psU0[0], psU0[1])
            if NQC > 1 and n < N_STORE:
                store01.append((n, attn_qk_exp(0, 1, n, pt_tag="pT01",
                                               pt_bufs=N_STORE)))
            if kts[-1] == SK_T - 1:
                finish_block(0, 0, psU0)

        # ---- remaining attention with projection fillers ----
        blocks = [(0, qc) for qc in range(1, NQC)]
        blocks += [(j, qc) for j in range(1, DT) for qc in range(NQC)]
        owed = {blk: [] for blk in blocks}
        for (j, qc) in blocks:
            if (j, qc) != (0, 1):
                owed[(j, qc)].append(lambda j=j, qc=qc: qproj(j, qc))
            if qc == 0 and j >= 1:
                for n in range(NCH):
                    owed[(j, qc)].append(lambda j=j, n=n: kproj(j, n))
        for f in owed[blocks[0]]:
            f()
        for bi, (j, qc) in enumerate(blocks):
            fillers = []
            if bi + 1 < len(blocks):
                fillers += owed[blocks[bi + 1]]
            last = bi == len(blocks) - 1
            if last and NQC > 1:
                # qc0's O is complete after block (DT-1, 0): transpose it
                # and run its output projection inside this last block
                for qt in range(SQ_T // NQC):
                    fillers.append(lambda qt=qt: transpose_qt(qt))
                    fillers.append(lambda qt=qt: outproj(qt))
            psU = new_psU()
            if (j, qc) == (0, 1) and store01:
                attn_span(j, qc, list(range(len(store01), SK_T)), psU,
                          fillers, precomputed=store01)
            else:
                attn_span(j, qc, list(range(SK_T)), psU, fillers)

        # ---- tail: last q-chunk's transposes + output rows ----
        qt_lo = SQ_T // NQC if NQC > 1 else 0
        for qt in range(qt_lo, SQ_T):
            transpose_qt(qt)
        for qt in range(qt_lo, SQ_T):
            outproj(qt, tail=True)


_NC_CACHE = {}


def _get_nc(sk=S, sq=SQ_FULL, skip_vbias=False):
    key = (sk, sq, skip_vbias)
    if key not in _NC_CACHE:
        _NC_CACHE[key] = build_mha(sk, sq, skip_vbias)
    return _NC_CACHE[key]


def _tile_rows(a):
    """[D, n] -> SBUF layout [P, DT*n]: partition p gets rows p, 128+p, ..."""
    Dd, n = a.shape
    t = Dd // P
    return np.ascontiguousarray(
        a.reshape(t, P, n).transpose(1, 0, 2).reshape(P, t * n))


def _tile_chunks(a, chp):
    """[D, sk] -> chunk-major SBUF layout [P, NCH*DT*chp]."""
    Dd, sk = a.shape
    t, nch = Dd // P, sk // chp
    return np.ascontiguousarray(
        a.reshape(t, P, nch, chp).transpose(1, 2, 0, 3).reshape(P, -1))


def prep_inputs(x, Wq, bq, Wk, bk, Wv, bv, Wo, bo):
    """Host-side sharding/layout prep: bf16 casts, feature-major transpose,
    SBUF pre-tiling.  Returns the 8 per-core input maps."""
    bf = ml_dtypes.bfloat16
    x = np.asarray(x, dtype=np.float32)
    shared = {
        "wq": _tile_rows(np.asarray(Wq, np.float32).astype(bf)),
        "wk": _tile_rows(np.asarray(Wk, np.float32).astype(bf)),
        "wv": _tile_rows(np.asarray(Wv, np.float32).astype(bf)),
        "wo": _tile_rows(np.asarray(Wo, np.float32).astype(bf)),
        "bq": np.ascontiguousarray(
            np.asarray(bq, np.float32).reshape(DT, P).T),
        "bk": np.ascontiguousarray(
            np.asarray(bk, np.float32).reshape(DT, P).T),
        "bv": np.asarray(bv, np.float32).astype(bf).reshape(1, D),
        "bo": np.asarray(bo, np.float32).astype(bf).reshape(1, D),
        "ident": np.eye(P, dtype=np.float32).astype(bf),
    }
    xT_b = [x[b].T.astype(bf) for b in range(B)]
    xT_tiled = [_tile_chunks(xb, 4 * P) for xb in xT_b]
    in_maps = []
    for c in range(NCORES):
        b, qo = divmod(c, QSPLIT)
        m = dict(shared)
        m["xT_bf"] = xT_tiled[b]
        m["xqT_bf"] = _tile_rows(
            xT_b[b][:, qo * SQ_FULL:(qo + 1) * SQ_FULL])
        in_maps.append(m)
    return in_maps


def kernel(x, Wq, bq, Wk, bk, Wv, bv, Wo, bo, **run_kwargs):
    """Full-input entry point: shards across 8 NeuronCores, returns full out."""
    in_maps = prep_inputs(x, Wq, bq, Wk, bk, Wv, bv, Wo, bo)
    nc = _get_nc(skip_vbias=bool(np.all(np.asarray(bv) == 0)))
    res = bass_utils.run_bass_kernel_spmd(
        nc, in_maps, core_ids=list(range(NCORES)), **run_kwargs)
    full = np.empty((B, S, D), np.float32)
    for c in range(NCORES):
        b, qo = divmod(c, QSPLIT)
        full[b, qo * SQ_FULL:(qo + 1) * SQ_FULL] = res.results[c]["out"]
    if run_kwargs:
        return full, res
    return full


# revision 18
# speedup vs baseline: 1.2178x; 1.0376x over previous
"""Trainium2 Bass kernel for nn_MultiHeadAttention (B=2, S=4096, D=512, H=8).

Computes: q/k/v = relu(x@W+b) per head, softmax(q k^T / sqrt(64)) v,
out = relu(concat_heads @ Wo + bo).

Sharding: 8 cores = 2 (batch) x 4 (query-slice).  Each core computes full
K/V projections for its batch (redundant across the 4 q-slice cores) and
attention + output projection for its 1024-row query slice.  No collectives;
the host concatenates the 8 output slices.

All-bf16 "flipped-PV" scheme (v3):
  - Projections and QK scores as in v1 (feature-major lhsT, paired heads in
    PE row groups 0/64, bias+relu fused on DVE, exp on ACT with scale 1/8).
  - The PV matmul is FLIPPED: out[128 queries, 65] with lhsT = pT slice
    [128 keys, 128 q] and rhs = V_pad [128 keys, 64+ones].  Matmul cost is
    the out free dim (65 cycles) and the per-matmul LDWEIGHTS streams 2
    rows/cycle, fully hidden - so PV drops from 512 to ~65 cycles per
    (ktile, head, qtile): 262k -> 133k PE cycles total.
  - Four [128, 65] accumulator regions share one PSUM bank per head
    (psU [128, 4, 65]); hardware start=True zeroing is bank-granular, so
    the bank is DVE-memset once per block and all PV matmuls accumulate
    with start=False (skip_group_check).
  - The ones column makes psU[:, qt, 64] the softmax denominator per query
    IN THE QUERY PARTITION: normalize is one tiny DVE reciprocal [128, 4]
    plus per-qtile tensor_scalar multiplies - no [1,512] reciprocal, no
    gpsimd broadcast, no deferred normalize queue.
  - O lands query-major; a PE transpose (identity matmul) per [128, 128]
    block rebuilds feature-major O^T for the output projection.  qc0's
    transposes+outproj run as fillers inside the last attention block.
"""

import os

import numpy as np
import ml_dtypes

import concourse.bass as bass
import concourse.mybir as mybir
import concourse.tile as tile
from concourse import bacc
from concourse import bass_utils

F32 = mybir.dt.float32
BF16 = mybir.dt.bfloat16
AF = mybir.ActivationFunctionType
ALU = mybir.AluOpType

P = 128
D = 512
H = 8
DH = 64
DT = D // P  # 4 (also = number of head pairs)
B = 2
S = 4096
NCORES = 8
QSPLIT = 4
SQ_FULL = S // QSPLIT  # 1024 query rows per core
QC = 512               # q-chunk (matmul free dim / PSUM bank width)
QT_C = QC // P         # 4 qtiles per q-chunk


def build_mha(sk=S, sq=SQ_FULL, skip_vbias=False):
    """Build the SPMD Bass program (identical on all cores)."""
    nc = bacc.Bacc("TRN2", target_bir_lowering=False, debug=False,
                   num_devices=NCORES)

    xT_d = nc.dram_tensor("xT_bf", (P, DT * sk), BF16,
                          kind="ExternalInput").ap()  # chunk-major, see prep
    xqT_d = nc.dram_tensor("xqT_bf", (P, DT * sq), BF16,
                           kind="ExternalInput").ap()
    w_dram = {}
    for n in ("wq", "wk", "wv", "wo"):
        w_dram[n] = nc.dram_tensor(n, (P, DT * D), BF16,
                                   kind="ExternalInput").ap()
    b_dram = {
        "bq": nc.dram_tensor("bq", (P, DT), F32, kind="ExternalInput").ap(),
        "bk": nc.dram_tensor("bk", (P, DT), F32, kind="ExternalInput").ap(),
        "bv": nc.dram_tensor("bv", (1, D), BF16, kind="ExternalInput").ap(),
        "bo": nc.dram_tensor("bo", (1, D), BF16, kind="ExternalInput").ap(),
    }
    ident_d = nc.dram_tensor("ident", (P, P), BF16, kind="ExternalInput").ap()
    out = nc.dram_tensor("out", (sq, D), F32, kind="ExternalOutput").ap()

    with tile.TileContext(nc) as tc:
        _build_tile(tc, xT_d, xqT_d, w_dram, b_dram, ident_d, out, sk, sq,
                    skip_vbias)

    nc.compile()
    return nc


def _build_tile(tc, xT_d, xqT_d, w_dram, b_dram, ident_d, out, sk, sq,
                skip_vbias=False):
    nc = tc.nc
    SK_T = sk // P            # ktiles of the key/value sequence
    SQ_T = sq // P
    NQC = sq // QC            # q chunks per core
    CH = min(4, SK_T)         # stiles per projection chunk
    NCH = SK_T // CH

    with (
        tc.tile_pool(name="singles", bufs=1) as singles,
        tc.tile_pool(name="work", bufs=3) as work,
        tc.tile_pool(name="psum", bufs=2, space="PSUM") as psum,
    ):
        # ---- startup: only what Q-proj pair 0 needs, first ----
        w_bf = {}
        w_bf["wq"] = singles.tile([P, DT, D], BF16, name="wq_bf")
        nc.sync.dma_start(w_bf["wq"], w_dram["wq"].rearrange(
            "p (t n) -> p t n", t=DT))
        b_col = {}
        b_col["bq"] = singles.tile([P, DT], F32, name="bq_col")
        nc.sync.dma_start(b_col["bq"], b_dram["bq"])
        xTq = singles.tile([P, DT, sq], BF16)
        xqT_src = xqT_d.rearrange("p (t s) -> p t s", t=DT)
        nc.sync.dma_start(xTq[:, :, 0:QC], xqT_src[:, :, 0:QC])
        w_bf["wk"] = singles.tile([P, DT, D], BF16, name="wk_bf")
        nc.sync.dma_start(w_bf["wk"], w_dram["wk"].rearrange(
            "p (t n) -> p t n", t=DT))
        b_col["bk"] = singles.tile([P, DT], F32, name="bk_col")
        nc.sync.dma_start(b_col["bk"], b_dram["bk"])
        CHP = CH * P
        NCH_ = (sk // P) // CH
        xT = singles.tile([P, NCH_, DT, CHP], BF16)
        xT_src = xT_d.rearrange("p (n t s) -> p n t s", n=NCH_, t=DT)
        nc.sync.dma_start(xT[:, 0], xT_src[:, 0])
        if sq > QC:
            nc.sync.dma_start(xTq[:, :, QC:], xqT_src[:, :, QC:])

        QT = singles.tile([P, DT, sq], BF16)

        def qproj(j, nq):
            psQ = psum.tile([P, QC], F32, tag="proj", bufs=1, name="psQ")
            for kt in range(DT):
                nc.tensor.matmul(
                    psQ, w_bf["wq"][:, kt, j * P:(j + 1) * P],
                    xTq[:, kt, nq * QC:(nq + 1) * QC],
                    start=(kt == 0), stop=(kt == DT - 1))
            nc.vector.tensor_scalar(
                QT[:, j, nq * QC:(nq + 1) * QC], psQ,
                b_col["bq"][:, j:j + 1], 0.0, op0=ALU.add, op1=ALU.max)

        qproj(0, 0)
        if NQC > 1:
            qproj(0, 1)

        # ---- K-proj deps next (attention can start before V exists) ----
        b_row = {}
        for n in ("wv", "wo"):
            wb = singles.tile([P, DT, D], BF16, name=f"{n}_bf")
            nc.sync.dma_start(wb, w_dram[n].rearrange(
                "p (t n) -> p t n", t=DT))
            w_bf[n] = wb
            if n == "wv":
                br = singles.tile([1, D], BF16, name="bv_row")
                nc.sync.dma_start(br, b_dram["bv"])
                b_row["bv"] = br
        br = singles.tile([1, D], BF16, name="bo_row")
        nc.sync.dma_start(br, b_dram["bo"])
        b_row["bo"] = br
        ident = singles.tile([P, P], BF16, name="ident")
        nc.sync.dma_start(ident, ident_d)

        # ---- persistent SBUF tensors ----
        xT1 = singles.tile([1, sk], BF16)
        nc.vector.memset(xT1, 1.0)
        KT = singles.tile([P, DT, sk], BF16)
        V_pad = singles.tile([P, SK_T, H, DH + 1], BF16)
        nc.vector.memset(V_pad[:, :, :, DH:DH + 1], 1.0)
        O_nat = singles.tile([P, SQ_T, H, DH], BF16)   # query-major O
        OT = singles.tile([P, DT, sq], BF16)           # feature-major O^T
        OT1 = singles.tile([1, sq], BF16)
        nc.vector.memset(OT1, 1.0)

        # PSUM: proj 2x1 + scores 2x2 + psU 2x(1040B) + tpose 1x(256B)
        def vproj(st):
            n, si = st // CH, st % CH
            psV = psum.tile([P, D], F32, tag="proj", bufs=1, name="psV")
            for kt in range(DT):
                nc.tensor.matmul(
                    psV, xT[:, n, kt, si * P:(si + 1) * P],
                    w_bf["wv"][:, kt, :],
                    start=(kt == 0),
                    stop=(skip_vbias and kt == DT - 1))
            if not skip_vbias:
                nc.tensor.matmul(psV, xT1[:, st * P:(st + 1) * P],
                                 b_row["bv"], start=False, stop=True)
            nc.vector.tensor_scalar_max(
                V_pad[:, st, :, 0:DH],
                psV.rearrange("p (h d) -> p h d", h=H), 0.0)

        def kproj(j, n):
            psK = psum.tile([P, CH * P], F32, tag="proj", bufs=1, name="psK")
            for kt in range(DT):
                nc.tensor.matmul(
                    psK, w_bf["wk"][:, kt, j * P:(j + 1) * P],
                    xT[:, n, kt, :],
                    start=(kt == 0), stop=(kt == DT - 1))
            nc.vector.tensor_scalar(
                KT[:, j, n * CH * P:(n + 1) * CH * P], psK,
                b_col["bk"][:, j:j + 1], 0.0, op0=ALU.add, op1=ALU.max)

        def attn_qk_exp(j, qc, kt, pt_tag="pT", pt_bufs=5):
            """Scores + exp for one ktile x 2 heads -> one ACT op."""
            q0 = qc * QC
            psS = psum.tile([P, 2 * QC], F32, tag="scores", bufs=2,
                            name="psS")
            nc.tensor.matmul(
                psS[:, 0:QC],
                KT[0:DH, j, kt * P:(kt + 1) * P],
                QT[0:DH, j, q0:q0 + QC], start=True, stop=True)
            nc.tensor.matmul(
                psS[:, QC:2 * QC],
                KT[DH:P, j, kt * P:(kt + 1) * P],
                QT[DH:P, j, q0:q0 + QC], start=True, stop=True)
            pT = work.tile([P, 2 * QC], BF16, tag=pt_tag,
                           bufs=pt_bufs, name="pT")
            nc.scalar.activation(pT, psS, AF.Exp, scale=0.125)
            return pT

        def attn_u(j, kt, pT, psU_A, psU_B):
            """Flipped PV: out[128 q, 65] per (head, qtile); start=False
            always (bank pre-zeroed by DVE), 65-cycle matmuls with hidden
            per-matmul LDWEIGHTS."""
            last = kt == SK_T - 1
            for h, psU in ((0, psU_A), (1, psU_B)):
                for qt in range(QT_C):
                    nc.tensor.matmul(
                        psU[:, qt, :],
                        pT[:, h * QC + qt * P:h * QC + (qt + 1) * P],
                        V_pad[:, kt, 2 * j + h, :],
                        start=False, stop=last, skip_group_check=True)

        def new_psU():
            """Two bank-packed accumulators (one per head), DVE-zeroed."""
            a = psum.tile([P, QT_C, DH + 1], F32, tag="psU", name="psU_A")
            b = psum.tile([P, QT_C, DH + 1], F32, tag="psU", name="psU_B")
            nc.vector.memset(a, 0.0)
            nc.vector.memset(b, 0.0)
            return (a, b)

        def finish_block(j, qc, psU):
            """Normalize straight out of PSUM: reciprocal of the 4 denom
            columns (per query partition!), then per-qtile scale into
            query-major O."""
            for h, psUh in enumerate(psU):
                rcp = work.tile([P, QT_C, 1], F32, tag="rcp", bufs=4,
                                name="rcp")
                nc.vector.reciprocal(rcp, psUh[:, :, DH:DH + 1])
                for qt in range(QT_C):
                    nc.vector.tensor_scalar(
                        O_nat[:, qc * QT_C + qt, 2 * j + h, :],
                        psUh[:, qt, 0:DH], rcp[:, qt], None, op0=ALU.mult)

        def attn_group(j, qc, kt, psU_A, psU_B):
            pT = attn_qk_exp(j, qc, kt)
            attn_u(j, kt, pT, psU_A, psU_B)

        def attn_span(j, qc, kts, psU, fillers=(), precomputed=()):
            """Emit the kt groups of one attention block, sprinkling
            `fillers` between groups so the in-order PE/DVE do them inside
            this ACT-bound stretch."""
            fillers = list(fillers)
            for kt, pT in precomputed:
                attn_u(j, kt, pT, psU[0], psU[1])
            ngroups = len(kts)
            spacing = max(1, ngroups // (len(fillers) + 1))
            gi = 0
            for kt in kts:
                attn_group(j, qc, kt, psU[0], psU[1])
                gi += 1
                if fillers and gi % spacing == 0:
                    fillers.pop(0)()
            for f in fillers:
                f()
            if kts[-1] == SK_T - 1:
                finish_block(j, qc, psU)

        def transpose_qt(qt):
            """O_nat[:, qt] (query-major) -> OT columns via PE transpose."""
            for j in range(DT):
                tp = psum.tile([P, P], BF16, tag="tpose", bufs=1, name="tp")
                nc.tensor.transpose(tp, O_nat[:, qt, 2 * j:2 * j + 2, :],
                                    ident)
                nc.vector.tensor_copy(OT[:, j, qt * P:(qt + 1) * P], tp)

        def outproj(qt, tail=False):
            if tail:
                # scores banks are dead after the last exp: borrow a slot so
                # tail chains double-buffer instead of serializing on the
                # single proj bank
                psO = psum.tile([P, 2 * QC], F32, tag="scores", bufs=2,
                                name="psO_t")[:, 0:D]
            else:
                psO = psum.tile([P, D], F32, tag="proj", bufs=1, name="psO")
            nc.tensor.matmul(psO, OT1[:, qt * P:(qt + 1) * P],
                             b_row["bo"], start=True, stop=False)
            for j in range(DT):
                nc.tensor.matmul(psO, OT[:, j, qt * P:(qt + 1) * P],
                                 w_bf["wo"][:, j, :],
                                 start=False, stop=(j == DT - 1))
            o_sb = work.tile([P, D], F32, tag="osb", bufs=2, name="o_sb")
            nc.scalar.activation(o_sb, psO, AF.Relu)
            nc.sync.dma_start(out[qt * P:(qt + 1) * P, :], o_sb)

        # ---- chunk loop: x load + V proj + K proj(pair 0) + attn(0, 0) ----
        psU0 = new_psU()
        N_STORE = 8
        store01 = []
        for n in range(NCH):
            if n > 0:
                nc.sync.dma_start(xT[:, n], xT_src[:, n])
            kproj(0, n)
            kts = list(range(n * CH, (n + 1) * CH))
            # order for an unbroken PE stream (p-state ramp): vprojs cover
            # the kproj-DVE and exp latencies so QK/attn_u never stall
            pTs = []
            vproj(kts[0])
            vproj(kts[1])
            for kt in kts[:2]:
                pTs.append((kt, attn_qk_exp(0, 0, kt)))
            vproj(kts[2])
            vproj(kts[3])
            for kt in kts[2:]:
                pTs.append((kt, attn_qk_exp(0, 0, kt)))
            for (kt, pT) in pTs:
                attn_u(0, kt, pT, psU0[0], psU0[1])
            if NQC > 1 and n < N_STORE:
                store01.append((n, attn_qk_exp(0, 1, n, pt_tag="pT01",
                                               pt_bufs=N_STORE)))
            if kts[-1] == SK_T - 1:
                finish_block(0, 0, psU0)

        # ---- remaining attention with projection fillers ----
        blocks = [(0, qc) for qc in range(1, NQC)]
        blocks += [(j, qc) for j in range(1, DT) for qc in range(NQC)]
        owed = {blk: [] for blk in blocks}
        for (j, qc) in blocks:
            if (j, qc) != (0, 1):
                owed[(j, qc)].append(lambda j=j, qc=qc: qproj(j, qc))
            if qc == 0 and j >= 1:
                for n in range(NCH):
                    owed[(j, qc)].append(lambda j=j, n=n: kproj(j, n))
        for f in owed[blocks[0]]:
            f()
        for bi, (j, qc) in enumerate(blocks):
            fillers = []
            if bi + 1 < len(blocks):
                fillers += owed[blocks[bi + 1]]
            last = bi == len(blocks) - 1
            if last and NQC > 1:
                # qc0's O is complete after block (DT-1, 0): transpose it
                # and run its output projection inside this last block
                for qt in range(SQ_T // NQC):
                    fillers.append(lambda qt=qt: transpose_qt(qt))
                    fillers.append(lambda qt=qt: outproj(qt))
            psU = new_psU()
            if (j, qc) == (0, 1) and store01:
                attn_span(j, qc, list(range(len(store01), SK_T)), psU,
                          fillers, precomputed=store01)
            else:
                attn_span(j, qc, list(range(SK_T)), psU, fillers)

        # ---- tail: last q-chunk's transposes + output rows ----
        qt_lo = SQ_T // NQC if NQC > 1 else 0
        for qt in range(qt_lo, SQ_T):
            transpose_qt(qt)
        for qt in range(qt_lo, SQ_T):
            outproj(qt, tail=True)


_NC_CACHE = {}


def _get_nc(sk=S, sq=SQ_FULL, skip_vbias=False):
    key = (sk, sq, skip_vbias)
    if key not in _NC_CACHE:
        _NC_CACHE[key] = build_mha(sk, sq, skip_vbias)
    return _NC_CACHE[key]


def _tile_rows(a):
    """[D, n] -> SBUF layout [P, DT*n]: partition p gets rows p, 128+p, ..."""
    Dd, n = a.shape
    t = Dd // P
    return np.ascontiguousarray(
        a.reshape(t, P, n).transpose(1, 0, 2).reshape(P, t * n))


def _tile_chunks(a, chp):
    """[D, sk] -> chunk-major SBUF layout [P, NCH*DT*chp]."""
    Dd, sk = a.shape
    t, nch = Dd // P, sk // chp
    return np.ascontiguousarray(
        a.reshape(t, P, nch, chp).transpose(1, 2, 0, 3).reshape(P, -1))


def prep_inputs(x, Wq, bq, Wk, bk, Wv, bv, Wo, bo):
    """Host-side sharding/layout prep: bf16 casts, feature-major transpose,
    SBUF pre-tiling.  Returns the 8 per-core input maps."""
    bf = ml_dtypes.bfloat16
    x = np.asarray(x, dtype=np.float32)
    shared = {
        "wq": _tile_rows(np.asarray(Wq, np.float32).astype(bf)),
        "wk": _tile_rows(np.asarray(Wk, np.float32).astype(bf)),
        "wv": _tile_rows(np.asarray(Wv, np.float32).astype(bf)),
        "wo": _tile_rows(np.asarray(Wo, np.float32).astype(bf)),
        "bq": np.ascontiguousarray(
            np.asarray(bq, np.float32).reshape(DT, P).T),
        "bk": np.ascontiguousarray(
            np.asarray(bk, np.float32).reshape(DT, P).T),
        "bv": np.asarray(bv, np.float32).astype(bf).reshape(1, D),
        "bo": np.asarray(bo, np.float32).astype(bf).reshape(1, D),
        "ident": np.eye(P, dtype=np.float32).astype(bf),
    }
    xT_b = [x[b].T.astype(bf) for b in range(B)]
    xT_tiled = [_tile_chunks(xb, 4 * P) for xb in xT_b]
    in_maps = []
    for c in range(NCORES):
        b, qo = divmod(c, QSPLIT)
        m = dict(shared)
        m["xT_bf"] = xT_tiled[b]
        m["xqT_bf"] = _tile_rows(
            xT_b[b][:, qo * SQ_FULL:(qo + 1) * SQ_FULL])
        in_maps.append(m)
    return in_maps


def kernel(x, Wq, bq, Wk, bk, Wv, bv, Wo, bo, **run_kwargs):
    """Full-input entry point: shards across 8 NeuronCores, returns full out."""
    in_maps = prep_inputs(x, Wq, bq, Wk, bk, Wv, bv, Wo, bo)
    nc = _get_nc(skip_vbias=bool(np.all(np.asarray(bv) == 0)))
    res = bass_utils.run_bass_kernel_spmd(
        nc, in_maps, core_ids=list(range(NCORES)), **run_kwargs)
    full = np.empty((B, S, D), np.float32)
    for c in range(NCORES):
        b, qo = divmod(c, QSPLIT)
        full[b, qo * SQ_FULL:(qo + 1) * SQ_FULL] = res.results[c]["out"]
    if run_kwargs:
        return full, res
    return full
